# revision 1
# baseline (speedup 1.0000x reference)
"""Trainium2 Bass kernel for the pairwise-score attention + gated MLP encoding.

Computation (per batch element b, p=1024 tokens, d=256 features):
    A[i,j]  = wa.P_i + wb.P_j + (P_i*wc).P_j
    itr     = softmax_j(A) @ P
    cat     = [P, itr]
    z       = tanh(cat@w1+b1); r = sigmoid(cat@w2+b2); f = sigmoid(cat@w3+b3)
    out     = r*P + f*z

Sharding: data-parallel over batch across 8 NeuronCores (4 batch el / core).

Kernel structure per batch element (all fp32r matmuls, on-chip throughout):
  - P loaded natural-layout into fp32r tiles; P^T via single-pass fp32r PE
    transposes, pairs sharing one PSUM tile so DVE evacuations move [128,256].
  - Scores transposed: S^T[j,i] = sum_d PT[d,j]*PcT[d,i].  The wa.P_i term is
    constant along the softmax axis j and cancels -> never computed.  The
    wb.P_j term is per-partition here -> folded into the exp as an ACT bias
    (sb via GpSimd mul + DVE row-reduce).
  - exp on the scalar engine from a 2-bank PSUM tile (one ACTIVATE per
    128x1024; scores are O(+-4), no max-subtraction needed).
  - Attention computed directly in the transposed layout the MLP needs:
    itrT_raw[d,i] = sum_j P[j,d]*expS^T[j,i] (stationary=P chunk, moving
    N=512).  Softmax denominators via an all-ones stationary matmul whose
    output is replicated across partitions by construction, so the DVE
    normalize is one reciprocal + one multiply per d-chunk.
  - MLP transposed (out^T = (cat@w)^T) so b1/b2/b3 are per-partition ACT
    biases; sigmoid evaluated as 0.5+0.5*tanh(x/2) so every activation stays
    in the one "exp_and_others" ACT table set (no table reloads).
  - Gating fused to 3 scalar_tensor_tensor ops per d-chunk:
    out = (t2+1)*(P/2) + 0.5*[(t3+1)*z], PE-transposed back, stored
    contiguously.
  - Emission is software-pipelined across batch elements: batch b+1's
    P^T-transposes and scores are emitted inside batch b's dependency bubbles
    so the (in-order) PE never idles long enough for HAM to re-throttle.
"""

import os
import sys

if "/opt/trn_rl_repo" not in sys.path:
    sys.path.insert(0, "/opt/trn_rl_repo")

import numpy as np

import concourse.bass as bass
import concourse.mybir as mybir
import concourse.tile as tile
from concourse import bacc
from concourse.bass_utils import run_bass_kernel_spmd
from concourse.masks import make_identity

F32 = mybir.dt.float32
F32R = mybir.dt.float32r
AF = mybir.ActivationFunctionType
ALU = mybir.AluOpType
AXX = mybir.AxisListType

B, PLEN, D = 32, 1024, 256
N_CORES = 8
B_LOC = B // N_CORES  # batch elements per core

NJ = PLEN // 128  # 8 token chunks of 128
ND = D // 128     # 2 feature chunks of 128


def _emit(ctx, tc, P_in, w_att, w_mlp, b_mlp, out):
    nc = tc.nc
    ts = bass.ts

    const = ctx.enter_context(tc.tile_pool(name="const", bufs=1))
    pin = ctx.enter_context(tc.tile_pool(name="pin", bufs=2))
    ptp = ctx.enter_context(tc.tile_pool(name="ptp", bufs=2))
    pexp = ctx.enter_context(tc.tile_pool(name="pexp", bufs=1))
    pitr = ctx.enter_context(tc.tile_pool(name="pitr", bufs=2))
    pmlp = ctx.enter_context(tc.tile_pool(name="pmlp", bufs=2))
    pout = ctx.enter_context(tc.tile_pool(name="pout", bufs=1))
    ps_big = ctx.enter_context(tc.tile_pool(name="ps_big", bufs=3, space="PSUM"))
    ps_t2 = ctx.enter_context(tc.tile_pool(name="ps_t2", bufs=2, space="PSUM"))

    # ---- constants (once per core) ----
    ident = const.tile([128, 128], F32)
    make_identity(nc, ident)
    ident_r = const.tile([128, 128], F32R)
    nc.vector.tensor_copy(out=ident_r, in_=ident)
    ones_f = const.tile([128, 128], F32)
    nc.vector.memset(ones_f, 1.0)
    ones_r = const.tile([128, 128], F32R)
    nc.vector.tensor_copy(out=ones_r, in_=ones_f)

    wc_sb = []
    for dc in range(ND):
        wc = const.tile([128, 1], F32, tag=f"wc{dc}")
        nc.gpsimd.dma_start(out=wc,
                            in_=w_att[bass.ds(2 * D + dc * 128, 128)].unsqueeze(1))
        wc_sb.append(wc)
    # wb broadcast to all partitions: [128, 256] (for the sb reduction)
    wbb = const.tile([128, D], F32)
    _wbs = w_att[bass.ds(D, D)]
    nc.gpsimd.dma_start(
        out=wbb,
        in_=bass.AP(tensor=_wbs.tensor, offset=_wbs.offset,
                    ap=[[0, 128]] + list(_wbs.ap)),
    )

    # MLP weights: [512, 256] -> sbuf [128, 4(kc), 256], fp32r
    w_sb = []
    for wi in range(3):
        wt = const.tile([128, 4, D], F32R, tag=f"w{wi}")
        nc.gpsimd.dma_start(
            out=wt, in_=w_mlp[wi].rearrange("(kc k) d -> k kc d", k=128).bitcast(F32R))
        w_sb.append(wt)

    # biases, per dout-chunk [128,1]; for r/f (sigmoid-via-tanh) we need b/2
    b_sb = []  # b_sb[wi][dc]
    for wi in range(3):
        chunks = []
        for dc in range(ND):
            bt = const.tile([128, 1], F32, tag=f"b{wi}{dc}")
            nc.gpsimd.dma_start(out=bt,
                                in_=b_mlp[wi][bass.ds(dc * 128, 128)].unsqueeze(1))
            if wi > 0:
                bh = const.tile([128, 1], F32, tag=f"bh{wi}{dc}")
                nc.scalar.mul(out=bh, in_=bt, mul=0.5)
                bt = bh
            chunks.append(bt)
        b_sb.append(chunks)

    # ---- per-batch-element phases ----
    def phase_load(b, split=False):
        Pn = []
        for jc in range(NJ):
            t = pin.tile([128, D], F32R, tag=f"pn{jc}", name=f"pn{jc}")
            eng = nc.scalar if (split and jc % 2) else nc.sync
            eng.dma_start(out=t, in_=P_in[b, ts(jc, 128), :].bitcast(F32R))
            Pn.append(t)
        return Pn

    def phase_pt(b, Pn):
        # P^T via paired single-pass fp32r PE transposes
        PT = [ptp.tile([128, PLEN], F32R, tag=f"pt{dc}", name=f"PT{dc}")
              for dc in range(ND)]
        for dc in range(ND):
            for j2 in range(NJ // 2):
                pst = ps_t2.tile([128, 256], F32R, tag="pst", name="pst")
                nc.tensor.transpose(pst[:, 0:128], Pn[2 * j2][:, ts(dc, 128)],
                                    ident_r)
                nc.tensor.transpose(pst[:, 128:256], Pn[2 * j2 + 1][:, ts(dc, 128)],
                                    ident_r)
                nc.vector.tensor_copy(out=PT[dc][:, ts(j2, 256)], in_=pst)
        # PcT = PT * wc ; Ph = PT / 2 (for the gating)
        PcT = [ptp.tile([128, PLEN], F32R, tag=f"pct{dc}", name=f"PcT{dc}")
               for dc in range(ND)]
        Ph = [ptp.tile([128, PLEN], F32, tag=f"ph{dc}", name=f"Ph{dc}")
              for dc in range(ND)]
        for dc in range(ND):
            nc.vector.tensor_scalar_mul(out=PcT[dc], in0=PT[dc].bitcast(F32),
                                        scalar1=wc_sb[dc])
            nc.vector.tensor_scalar_mul(out=Ph[dc], in0=PT[dc].bitcast(F32),
                                        scalar1=0.5)
        # sb[j] = P_j . wb : GpSimd elementwise mul + DVE row-reduce
        sb_sb = []
        for jc in range(NJ):
            scr = pin.tile([128, D], F32, tag="sbscr", name="scr")
            s = pin.tile([128, 1], F32, tag=f"sbj{jc}", name=f"sbj{jc}")
            nc.gpsimd.tensor_mul(out=scr, in0=Pn[jc].bitcast(F32), in1=wbb)
            nc.vector.reduce_sum(out=s, in_=scr, axis=AXX.X)
            sb_sb.append(s)
        return PT, PcT, Ph, sb_sb

    def phase_scores(b, PT, PcT, sb_sb):
        expST = [pexp.tile([128, PLEN], F32R, tag=f"es{jc}", name=f"expST{jc}")
                 for jc in range(NJ)]
        for jc in range(NJ):
            pss = ps_big.tile([128, 1024], F32, tag="big", name="pss")
            for ic2 in range(2):
                nc.tensor.matmul(pss[:, ts(ic2, 512)], PT[0][:, ts(jc, 128)],
                                 PcT[0][:, ts(ic2, 512)], start=True, stop=False)
                nc.tensor.matmul(pss[:, ts(ic2, 512)], PT[1][:, ts(jc, 128)],
                                 PcT[1][:, ts(ic2, 512)], start=False, stop=True)
            nc.scalar.activation(out=expST[jc], in_=pss, func=AF.Exp,
                                 bias=sb_sb[jc], scale=1.0)
        return expST

    def phase_attn(b, Pn, expST):
        # softmax denominators, replicated across partitions by the all-ones
        # stationary operand
        psd = ps_big.tile([128, 1024], F32, tag="big", name="psd")
        for ic2 in range(2):
            for jc in range(NJ):
                nc.tensor.matmul(psd[:, ts(ic2, 512)], ones_r,
                                 expST[jc][:, ts(ic2, 512)],
                                 start=(jc == 0), stop=(jc == NJ - 1))
        # itr^T numerator, directly in the layout the MLP consumes
        psum_it = []
        for dc in range(ND):
            pit = ps_big.tile([128, 1024], F32, tag="big", name=f"pit{dc}")
            for ic2 in range(2):
                for jc in range(NJ):
                    nc.tensor.matmul(pit[:, ts(ic2, 512)], Pn[jc][:, ts(dc, 128)],
                                     expST[jc][:, ts(ic2, 512)],
                                     start=(jc == 0), stop=(jc == NJ - 1))
            psum_it.append(pit)
        recipb = pitr.tile([128, PLEN], F32, tag="recipb", name="recipb")
        nc.vector.reciprocal_approx_fast(out=recipb, in_=psd)
        itrT = [pitr.tile([128, PLEN], F32R, tag=f"it{dc}", name=f"itrT{dc}")
                for dc in range(ND)]
        for dc in range(ND):
            nc.vector.tensor_mul(out=itrT[dc], in0=psum_it[dc], in1=recipb)
        return itrT

    def phase_mlp(b, PT, itrT, Ph):
        catT = [PT[0], PT[1], itrT[0], itrT[1]]
        oT = []
        for dc in range(ND):
            acts = []
            for wi in range(3):
                psm = ps_big.tile([128, 1024], F32, tag="big", name="psm")
                for pc in range(2):
                    for kc in range(4):
                        nc.tensor.matmul(
                            psm[:, ts(pc, 512)],
                            w_sb[wi][:, kc, ts(dc, 128)],
                            catT[kc][:, ts(pc, 512)],
                            start=(kc == 0), stop=(kc == 3),
                        )
                t = pmlp.tile([128, PLEN], F32, tag=f"act{wi}", name=f"act{wi}")
                if wi == 0:
                    nc.scalar.activation(out=t, in_=psm, func=AF.Tanh,
                                         bias=b_sb[0][dc], scale=1.0)
                else:
                    nc.scalar.activation(out=t, in_=psm, func=AF.Tanh,
                                         bias=b_sb[wi][dc], scale=0.5)
                acts.append(t)
            z_t, t2, t3 = acts
            # out^T = (t2+1)*(P/2) + 0.5*[(t3+1)*z], in p-halves so the
            # output transposes can start after the first half
            o = pmlp.tile([128, PLEN], F32R, tag=f"oT{dc}", name=f"oT{dc}")
            for pc in range(2):
                sl = ts(pc, 512)
                m1 = pmlp.tile([128, 512], F32, tag="m1", name="m1", bufs=1)
                nc.vector.scalar_tensor_tensor(out=m1, in0=t2[:, sl], scalar=1.0,
                                               in1=Ph[dc][:, sl],
                                               op0=ALU.add, op1=ALU.mult)
                m2 = pmlp.tile([128, 512], F32, tag="m2", name="m2", bufs=1)
                nc.vector.scalar_tensor_tensor(out=m2, in0=t3[:, sl], scalar=1.0,
                                               in1=z_t[:, sl],
                                               op0=ALU.add, op1=ALU.mult)
                nc.vector.scalar_tensor_tensor(out=o[:, sl], in0=m2, scalar=0.5,
                                               in1=m1, op0=ALU.mult, op1=ALU.add)
            oT.append(o)
        return oT

    def phase_out(b, oT):
        for p2 in range(NJ):
            onat = pout.tile([128, D], F32, tag=f"on{p2}", name=f"onat{p2}")
            pst = ps_t2.tile([128, 256], F32R, tag="pst", name="pst")
            nc.tensor.transpose(pst[:, 0:128], oT[0][:, ts(p2, 128)], ident_r)
            nc.tensor.transpose(pst[:, 128:256], oT[1][:, ts(p2, 128)], ident_r)
            nc.vector.tensor_copy(out=onat, in_=pst)
            nc.sync.dma_start(out=out[b, ts(p2, 128), :], in_=onat)

    # ---- software-pipelined emission across batch elements ----
    # PE order per iteration: attn(b) | out(b-1) | pt(b+1) | mlp(b) |
    # scores(b+1) -- the out/pt phases fill the attn->mlp dependency bubble
    # (itrT normalization on DVE) so the in-order PE never idles long enough
    # for HAM to re-throttle, including on the final batch element.
    Pn = phase_load(0, split=True)
    PT, PcT, Ph, sb_sb = phase_pt(0, Pn)
    expST = phase_scores(0, PT, PcT, sb_sb)
    oT_prev = None
    for b in range(B_LOC):
        if b + 1 < B_LOC:
            Pn_n = phase_load(b + 1)
        itrT = phase_attn(b, Pn, expST)
        if oT_prev is not None:
            phase_out(b - 1, oT_prev)
        if b + 1 < B_LOC:
            PT_n, PcT_n, Ph_n, sb_n = phase_pt(b + 1, Pn_n)
        oT = phase_mlp(b, PT, itrT, Ph)
        if b + 1 < B_LOC:
            expST = phase_scores(b + 1, PT_n, PcT_n, sb_n)
        oT_prev = oT
        if b + 1 < B_LOC:
            Pn, PT, PcT, Ph = Pn_n, PT_n, PcT_n, Ph_n
    phase_out(B_LOC - 1, oT_prev)


_NC_CACHE = {}


def _build():
    if "nc" in _NC_CACHE:
        return _NC_CACHE["nc"]
    nc = bacc.Bacc("TRN2", target_bir_lowering=False, debug=False,
                   num_devices=N_CORES)
    P_in = nc.dram_tensor("p_in", [B_LOC, PLEN, D], F32, kind="ExternalInput").ap()
    w_att = nc.dram_tensor("w_att", [3 * D], F32, kind="ExternalInput").ap()
    w_mlp = [nc.dram_tensor(f"w{i}", [2 * D, D], F32, kind="ExternalInput").ap()
             for i in (1, 2, 3)]
    b_mlp = [nc.dram_tensor(f"b{i}", [D], F32, kind="ExternalInput").ap()
             for i in (1, 2, 3)]
    out = nc.dram_tensor("out", [B_LOC, PLEN, D], F32, kind="ExternalOutput").ap()

    from contextlib import ExitStack

    with tile.TileContext(nc) as tc, ExitStack() as ctx:
        _emit(ctx, tc, P_in, w_att, w_mlp, b_mlp, out)
    nc.compile()
    _NC_CACHE["nc"] = nc
    return nc


def run(inputs, trace=False, tmpdir=None):
    nc = _build()
    P = np.ascontiguousarray(np.asarray(inputs["P"], dtype=np.float32))
    shared = {
        "w_att": np.ascontiguousarray(np.asarray(inputs["w_itr_att"], np.float32)),
        "w1": np.ascontiguousarray(np.asarray(inputs["w1"], np.float32)),
        "w2": np.ascontiguousarray(np.asarray(inputs["w2"], np.float32)),
        "w3": np.ascontiguousarray(np.asarray(inputs["w3"], np.float32)),
        "b1": np.ascontiguousarray(np.asarray(inputs["b1"], np.float32)),
        "b2": np.ascontiguousarray(np.asarray(inputs["b2"], np.float32)),
        "b3": np.ascontiguousarray(np.asarray(inputs["b3"], np.float32)),
    }
    in_maps = [
        {"p_in": P[c * B_LOC : (c + 1) * B_LOC], **shared} for c in range(N_CORES)
    ]
    res = run_bass_kernel_spmd(nc, in_maps, list(range(N_CORES)), trace=trace,
                               tmpdir=tmpdir)
    full = np.concatenate([res.results[c]["out"] for c in range(N_CORES)], axis=0)
    return full, res


def kernel(**inputs):
    full, _ = run(inputs)
    return full



# revision 5
# speedup vs baseline: 1.1668x; 1.1668x over previous
"""Trainium2 Bass kernel for the pairwise-score attention + gated MLP encoding.

Computation (per batch element b, p=1024 tokens, d=256 features):
    A[i,j]  = wa.P_i + wb.P_j + (P_i*wc).P_j
    itr     = softmax_j(A) @ P
    cat     = [P, itr]
    z       = tanh(cat@w1+b1); r = sigmoid(cat@w2+b2); f = sigmoid(cat@w3+b3)
    out     = r*P + f*z

Sharding: data-parallel over batch across 8 NeuronCores (4 batch el / core).

v2 kernel structure (per batch element):
  - P shipped bf16 from host.  P^T produced by XBAR DMA transposes straight
    from DRAM (no PE transposes, no evacuations).  The wa.P_i term is constant
    along the softmax axis j and cancels -> never computed.
  - sb[j] = P_j.wb via 16 tiny PE matvecs off P^T (N=1), evacuated once as a
    [128,8] bias tile; folded into the exp as an ACT bias.
  - Score + attention matmuls run in fp8e4 DoubleRow (2 fp8 weights/PE cell,
    K=256 per pass -> 0.5 cyc/row): S^T = PT8.T @ PcT8 per j-chunk,
    exp on ACT writes fp8 directly, softmax denominator via an all-ones fp8
    stationary, itr^T numerator with stationary Pn8 j-chunk pairs.
  - MLP in bf16, transposed (out^T = (cat@w)^T) so b1/b2/b3 are per-partition
    ACT biases; sigmoid as 0.5+0.5*tanh(x/2) keeps one ACT table set.
  - Gating fully bf16 (DVE 2x/4x modes): out = (t2+1)*(P/2)+0.5*[(t3+1)*z].
  - Output transposed back on the PE in bf16, stored bf16; host casts to f32.
  - Emission software-pipelined across batch elements as in v1 so the PE and
    ACT never idle long enough for HAM to re-throttle.
"""

import sys

if "/opt/trn_rl_repo" not in sys.path:
    sys.path.insert(0, "/opt/trn_rl_repo")

import numpy as np
import ml_dtypes

import concourse.bass as bass
import concourse.mybir as mybir
import concourse.tile as tile
from concourse import bacc
from concourse.bass_utils import run_bass_kernel_spmd
from concourse.masks import make_identity

F32 = mybir.dt.float32
BF16 = mybir.dt.bfloat16
FP8 = mybir.dt.float8e4
AF = mybir.ActivationFunctionType
ALU = mybir.AluOpType
DRM = mybir.MatmulPerfMode.DoubleRow

B, PLEN, D = 32, 1024, 256
N_CORES = 8
B_LOC = B // N_CORES  # batch elements per core

NJ = PLEN // 128  # 8 token chunks of 128
ND = D // 128     # 2 feature chunks of 128
NPAIR = NJ // 2   # 4 token-chunk pairs (fp8 DoubleRow K=256)


def _emit(ctx, tc, P_in, wb_in, wc_in, w_mlp, b_mlp, out):
    nc = tc.nc
    ts = bass.ts

    const = ctx.enter_context(tc.tile_pool(name="const", bufs=1))
    pin = ctx.enter_context(tc.tile_pool(name="pin", bufs=2))
    ptp = ctx.enter_context(tc.tile_pool(name="ptp", bufs=2))
    pexp = ctx.enter_context(tc.tile_pool(name="pexp", bufs=2))
    pitr = ctx.enter_context(tc.tile_pool(name="pitr", bufs=2))
    pmlp = ctx.enter_context(tc.tile_pool(name="pmlp", bufs=2))
    pout = ctx.enter_context(tc.tile_pool(name="pout", bufs=1))
    # PSUM is 8 banks: ps_big 3x2 + pst 1 + psb 1
    ps_big = ctx.enter_context(tc.tile_pool(name="ps_big", bufs=3, space="PSUM"))
    ps_t2 = ctx.enter_context(tc.tile_pool(name="ps_t2", bufs=1, space="PSUM"))
    ps_sb = ctx.enter_context(tc.tile_pool(name="ps_sb", bufs=1, space="PSUM"))

    as3 = lambda ap: ap.rearrange("p (c x) -> p c x", c=2)

    # ---- constants (once per core) ----
    ident = const.tile([128, 128], F32)
    make_identity(nc, ident)
    ident_bf = const.tile([128, 128], BF16)
    nc.vector.tensor_copy(out=ident_bf, in_=ident)
    ones_f = const.tile([128, 256], F32)
    nc.vector.memset(ones_f, 1.0)
    ones8 = const.tile([128, 256], FP8)  # as3 -> [128, 2, 128] DR stationary
    nc.vector.tensor_copy(out=ones8, in_=ones_f)

    # wb (bf16) as [128,1] per d-chunk (matvec moving operand)
    wb_sb = []
    for dc in range(ND):
        t = const.tile([128, 1], BF16, tag=f"wb{dc}")
        nc.gpsimd.dma_start(out=t, in_=wb_in[bass.ds(dc * 128, 128)].unsqueeze(1))
        wb_sb.append(t)
    # wc (f32) as [128,1] per d-chunk (tensor_scalar operand)
    wc_sb = []
    for dc in range(ND):
        t = const.tile([128, 1], F32, tag=f"wc{dc}")
        nc.gpsimd.dma_start(out=t, in_=wc_in[bass.ds(dc * 128, 128)].unsqueeze(1))
        wc_sb.append(t)

    # MLP weights (bf16): [512, 256] -> sbuf [128, 4(kc), 256]
    w_sb = []
    for wi in range(3):
        wt = const.tile([128, 4, D], BF16, tag=f"w{wi}")
        nc.gpsimd.dma_start(
            out=wt, in_=w_mlp[wi].rearrange("(kc k) d -> k kc d", k=128))
        w_sb.append(wt)

    # biases, per dout-chunk [128,1]; for r/f (sigmoid-via-tanh) we need b/2
    b_sb = []  # b_sb[wi][dc]
    for wi in range(3):
        chunks = []
        for dc in range(ND):
            bt = const.tile([128, 1], F32, tag=f"b{wi}{dc}")
            nc.gpsimd.dma_start(out=bt,
                                in_=b_mlp[wi][bass.ds(dc * 128, 128)].unsqueeze(1))
            if wi > 0:
                bh = const.tile([128, 1], F32, tag=f"bh{wi}{dc}")
                nc.scalar.mul(out=bh, in_=bt, mul=0.5)
                bt = bh
            chunks.append(bt)
        b_sb.append(chunks)

    # ---- per-batch-element phases ----
    def phase_load(b, split=False):
        """Natural-layout bf16 tiles + XBAR DMA transposes into PT."""
        Pn = []
        for jc in range(NJ):
            t = pin.tile([128, D], BF16, tag=f"pn{jc}", name=f"pn{jc}")
            eng = nc.scalar if (split and jc % 2) else nc.sync
            eng.dma_start(out=t, in_=P_in[b, ts(jc, 128), :])
            Pn.append(t)
        # PT[p, dc*1024 + j] = P[j, dc*128+p]
        PT = ptp.tile([128, 2 * PLEN], BF16, tag="pt", name="PT")
        for dc in range(ND):
            eng = nc.scalar if (split and dc % 2) else nc.sync
            eng.dma_start_transpose(out=PT[:, ts(dc, PLEN)],
                                    in_=P_in[b, :, ts(dc, 128)])
        return Pn, PT

    def phase_prep(b, Pn, PT):
        """sb matvecs on PE; fp8/bf16 derivatives on DVE."""
        psb = ps_sb.tile([128, NJ], F32, tag="psb", name="psb")
        for jc in range(NJ):
            for dc in range(ND):
                nc.tensor.matmul(psb[:, jc:jc + 1],
                                 PT[:, bass.ds(dc * PLEN + jc * 128, 128)],
                                 wb_sb[dc], start=(dc == 0), stop=(dc == ND - 1))
        sb = ptp.tile([128, NJ], F32, tag="sb", name="sb")
        # -2 shift keeps exp(A) well under the fp8e4 max of 240; it cancels
        # exactly in the softmax normalization
        nc.vector.tensor_scalar_add(out=sb, in0=psb, scalar1=-2.0)

        PT8 = ptp.tile([128, 2 * PLEN], FP8, tag="pt8", name="PT8")
        nc.vector.tensor_copy(out=PT8, in_=PT)
        PcT8 = ptp.tile([128, 2 * PLEN], FP8, tag="pct8", name="PcT8")
        for dc in range(ND):
            nc.vector.tensor_scalar_mul(out=PcT8[:, ts(dc, PLEN)],
                                        in0=PT[:, ts(dc, PLEN)],
                                        scalar1=wc_sb[dc])
        Ph = ptp.tile([128, 2 * PLEN], BF16, tag="ph", name="Ph")
        nc.vector.tensor_scalar_mul(out=Ph, in0=PT, scalar1=0.5)
        # Pn8 pairs: [128, 512] flat = [c(2) x d(256)] per j-chunk pair
        Pn8 = []
        for m in range(NPAIR):
            t = ptp.tile([128, 2 * D], FP8, tag=f"pn8{m}", name=f"Pn8{m}")
            for c in range(2):
                nc.vector.tensor_copy(out=t[:, ts(c, D)], in_=Pn[2 * m + c])
            Pn8.append(t)
        return PT8, PcT8, Ph, Pn8, sb

    def phase_scores(b, PT8, PcT8, sb):
        """S^T per j-chunk in fp8 DoubleRow; exp -> fp8 pair tiles."""
        exps = [pexp.tile([128, 2 * PLEN], FP8, tag=f"es{m}", name=f"expS{m}")
                for m in range(NPAIR)]
        for jc in range(NJ):
            pss = ps_big.tile([128, 1024], F32, tag="big", name="pss")
            for ic2 in range(2):
                nc.tensor.matmul(pss[:, ts(ic2, 512)],
                                 as3(PT8)[:, :, ts(jc, 128)],
                                 as3(PcT8)[:, :, ts(ic2, 512)],
                                 start=True, stop=True, perf_mode=DRM)
            nc.scalar.activation(out=exps[jc // 2][:, ts(jc % 2, PLEN)], in_=pss,
                                 func=AF.Exp, bias=sb[:, jc:jc + 1], scale=1.0)
        return exps

    def phase_attn(b, Pn8, exps):
        # softmax denominators, replicated across partitions by the all-ones
        # fp8 stationary
        psd = ps_big.tile([128, 1024], F32, tag="big", name="psd")
        for ic2 in range(2):
            for m in range(NPAIR):
                nc.tensor.matmul(psd[:, ts(ic2, 512)], as3(ones8),
                                 as3(exps[m])[:, :, ts(ic2, 512)],
                                 start=(m == 0), stop=(m == NPAIR - 1),
                                 perf_mode=DRM)
        # itr^T numerator, directly in the layout the MLP consumes
        psum_it = []
        for dc in range(ND):
            pit = ps_big.tile([128, 1024], F32, tag="big", name=f"pit{dc}")
            for ic2 in range(2):
                for m in range(NPAIR):
                    nc.tensor.matmul(pit[:, ts(ic2, 512)],
                                     as3(Pn8[m])[:, :, ts(dc, 128)],
                                     as3(exps[m])[:, :, ts(ic2, 512)],
                                     start=(m == 0), stop=(m == NPAIR - 1),
                                     perf_mode=DRM)
            psum_it.append(pit)
        recipb = pitr.tile([128, PLEN], F32, tag="recipb", name="recipb")
        nc.vector.reciprocal_approx_fast(out=recipb, in_=psd)
        itrT = [pitr.tile([128, PLEN], BF16, tag=f"it{dc}", name=f"itrT{dc}")
                for dc in range(ND)]
        for dc in range(ND):
            nc.vector.tensor_mul(out=itrT[dc], in0=psum_it[dc], in1=recipb)
        return itrT

    def phase_mlp(b, PT, itrT, Ph):
        catT = [PT[:, 0:PLEN], PT[:, PLEN:2 * PLEN], itrT[0], itrT[1]]
        oT = pmlp.tile([128, 2 * PLEN], BF16, tag="oT", name="oT")
        for dc in range(ND):
            acts = []
            for wi in range(3):
                psm = ps_big.tile([128, 1024], F32, tag="big", name="psm")
                for pc in range(2):
                    for kc in range(4):
                        nc.tensor.matmul(
                            psm[:, ts(pc, 512)],
                            w_sb[wi][:, kc, ts(dc, 128)],
                            catT[kc][:, ts(pc, 512)],
                            start=(kc == 0), stop=(kc == 3),
                        )
                t = pmlp.tile([128, PLEN], BF16, tag=f"act{wi}", name=f"act{wi}")
                nc.scalar.activation(out=t, in_=psm, func=AF.Tanh,
                                     bias=b_sb[wi][dc],
                                     scale=(1.0 if wi == 0 else 0.5))
                acts.append(t)
            z_t, t2, t3 = acts
            # out^T = (t2+1)*(P/2) + 0.5*[(t3+1)*z], in p-halves so the
            # output transposes can start after the first half
            for pc in range(2):
                sl = ts(pc, 512)
                osl = bass.ds(dc * PLEN + pc * 512, 512)
                m1 = pmlp.tile([128, 512], BF16, tag="m1", name="m1", bufs=1)
                nc.vector.scalar_tensor_tensor(out=m1, in0=t2[:, sl], scalar=1.0,
                                               in1=Ph[:, osl],
                                               op0=ALU.add, op1=ALU.mult)
                m2 = pmlp.tile([128, 512], BF16, tag="m2", name="m2", bufs=1)
                nc.vector.scalar_tensor_tensor(out=m2, in0=t3[:, sl], scalar=1.0,
                                               in1=z_t[:, sl],
                                               op0=ALU.add, op1=ALU.mult)
                nc.vector.scalar_tensor_tensor(out=oT[:, osl], in0=m2, scalar=0.5,
                                               in1=m1, op0=ALU.mult, op1=ALU.add)
        return oT

    def phase_out(b, oT):
        for p2 in range(NJ):
            onat = pout.tile([128, D], BF16, tag=f"on{p2}", name=f"onat{p2}")
            pst = ps_t2.tile([128, 256], BF16, tag="pst", name="pst")
            nc.tensor.transpose(pst[:, 0:128],
                                oT[:, bass.ds(0 * PLEN + p2 * 128, 128)], ident_bf)
            nc.tensor.transpose(pst[:, 128:256],
                                oT[:, bass.ds(1 * PLEN + p2 * 128, 128)], ident_bf)
            nc.vector.tensor_copy(out=onat, in_=pst)
            nc.sync.dma_start(out=out[b, ts(p2, 128), :], in_=onat)

    # ---- software-pipelined emission across batch elements ----
    # PE order per iteration: attn(b) | out(b-1) | prep(b+1) | mlp(b) |
    # scores(b+1) -- the out/prep phases fill the attn->mlp dependency bubble
    # (itrT normalization on DVE) so the in-order PE never idles long enough
    # for HAM to re-throttle, including on the final batch element.
    Pn, PT = phase_load(0, split=True)
    PT8, PcT8, Ph, Pn8, sb = phase_prep(0, Pn, PT)
    exps = phase_scores(0, PT8, PcT8, sb)
    oT_prev = None
    for b in range(B_LOC):
        if b + 1 < B_LOC:
            Pn_n, PT_n = phase_load(b + 1)
        itrT = phase_attn(b, Pn8, exps)
        if oT_prev is not None:
            phase_out(b - 1, oT_prev)
        if b + 1 < B_LOC:
            PT8_n, PcT8_n, Ph_n, Pn8_n, sb_n = phase_prep(b + 1, Pn_n, PT_n)
        oT = phase_mlp(b, PT, itrT, Ph)
        if b + 1 < B_LOC:
            exps = phase_scores(b + 1, PT8_n, PcT8_n, sb_n)
        oT_prev = oT
        if b + 1 < B_LOC:
            PT, Ph, Pn8 = PT_n, Ph_n, Pn8_n
    phase_out(B_LOC - 1, oT_prev)


_NC_CACHE = {}


def _build():
    if "nc" in _NC_CACHE:
        return _NC_CACHE["nc"]
    nc = bacc.Bacc("TRN2", target_bir_lowering=False, debug=False,
                   num_devices=N_CORES)
    P_in = nc.dram_tensor("p_in", [B_LOC, PLEN, D], BF16, kind="ExternalInput").ap()
    wb_in = nc.dram_tensor("wb", [D], BF16, kind="ExternalInput").ap()
    wc_in = nc.dram_tensor("wc", [D], F32, kind="ExternalInput").ap()
    w_mlp = [nc.dram_tensor(f"w{i}", [2 * D, D], BF16, kind="ExternalInput").ap()
             for i in (1, 2, 3)]
    b_mlp = [nc.dram_tensor(f"b{i}", [D], F32, kind="ExternalInput").ap()
             for i in (1, 2, 3)]
    out = nc.dram_tensor("out", [B_LOC, PLEN, D], BF16, kind="ExternalOutput").ap()

    from contextlib import ExitStack

    with tile.TileContext(nc) as tc, ExitStack() as ctx:
        _emit(ctx, tc, P_in, wb_in, wc_in, w_mlp, b_mlp, out)
    nc.compile()
    _NC_CACHE["nc"] = nc
    return nc


def run(inputs, trace=False, tmpdir=None):
    nc = _build()
    bf = ml_dtypes.bfloat16
    P = np.ascontiguousarray(np.asarray(inputs["P"], dtype=np.float32)).astype(bf)
    w_att = np.asarray(inputs["w_itr_att"], np.float32)
    shared = {
        "wb": np.ascontiguousarray(w_att[D:2 * D]).astype(bf),
        "wc": np.ascontiguousarray(w_att[2 * D:3 * D]),
        "w1": np.ascontiguousarray(np.asarray(inputs["w1"], np.float32)).astype(bf),
        "w2": np.ascontiguousarray(np.asarray(inputs["w2"], np.float32)).astype(bf),
        "w3": np.ascontiguousarray(np.asarray(inputs["w3"], np.float32)).astype(bf),
        "b1": np.ascontiguousarray(np.asarray(inputs["b1"], np.float32)),
        "b2": np.ascontiguousarray(np.asarray(inputs["b2"], np.float32)),
        "b3": np.ascontiguousarray(np.asarray(inputs["b3"], np.float32)),
    }
    in_maps = [
        {"p_in": P[c * B_LOC : (c + 1) * B_LOC], **shared} for c in range(N_CORES)
    ]
    res = run_bass_kernel_spmd(nc, in_maps, list(range(N_CORES)), trace=trace,
                               tmpdir=tmpdir)
    full = np.concatenate(
        [np.asarray(res.results[c]["out"]).astype(np.float32)
         for c in range(N_CORES)], axis=0)
    return full, res


def kernel(**inputs):
    full, _ = run(inputs)
    return full
